# revision 1
# baseline (speedup 1.0000x reference)
"""Capsule-routing kernel for Trainium2 (8 NeuronCores, Bass/Tile).

Problem: u[b,o,k,j] = sum_i x[b,k,i] W[o,k,i,j]; 3 iters of dynamic routing
(softmax over o of per-(b,k) logits, squash over j), output v[b,o,j].

Sharding: input-capsule (IC=2048) dim split across 8 cores (256 each).
Per core: u (8M elems) is produced by TensorE matmuls with a block-diagonal
x as the stationary operand and W streamed, kept resident in SBUF as bf16
in layout [p=(b16,k8), kg32, (o,j)1024] (two tensors, one per batch-half).
Routing weighted-sums over k run as (DVE multiply) + (TensorE delta-ones
reduction with PSUM accumulation); the per-(b,o,k) logit update (sum over j
of a*u) runs as DVE multiply + contiguous-halves add tree. The per-iteration
partial s [32,1024] is AllReduce'd across the 8 cores (routing softmax is
local: o lives in the free dim).

Shapes (hardcoded): B=32, IC=2048, IV=16, OC=32, OV=32, T=3.
"""

import sys

sys.path.insert(0, "/opt/trn_rl_repo")

import numpy as np
import ml_dtypes

import concourse.bass as bass
import concourse.mybir as mybir
import concourse.tile as tile

BF16 = mybir.dt.bfloat16
F32 = mybir.dt.float32

NCORES = 8
B, IC, IV, OC, OV = 32, 2048, 16, 32, 32
KL = IC // NCORES          # 256 local input capsules per core
KG = KL // 8               # 32 k-groups of 8
OJ = OC * OV               # 1024
T = 3

_BF = ml_dtypes.bfloat16


def build_nc(split_waits=True):
    nc = bass.Bass()

    w_in = nc.declare_dram_parameter("w", [KG, 128, OJ], BF16, isOutput=False)
    xblk_in = nc.declare_dram_parameter("xblk", [128, 2, KG, 128], BF16, isOutput=False)
    xsum_in = nc.declare_dram_parameter("xsum", [128, KG, 32], BF16, isOutput=False)
    dsum0_in = nc.declare_dram_parameter("dsum0", [2, 128, 32], BF16, isOutput=False)
    dsumA_in = nc.declare_dram_parameter("dsumA", [2, 128, 32], BF16, isOutput=False)
    drep_in = nc.declare_dram_parameter("drep", [2, 32, 128], BF16, isOutput=False)
    v_out = nc.declare_dram_parameter("v", [B, OJ], F32, isOutput=True)

    with tile.TileContext(nc) as tc:
        with nc.allow_low_precision(reason="bf16 routing intermediates"):
            _emit(nc, tc, w_in, xblk_in, xsum_in, dsum0_in, dsumA_in, drep_in, v_out)
    if split_waits:
        _split_excess_waits(nc)
    return nc


def _split_excess_waits(nc):
    """The walrus build in this container accepts very few sync-wait commands
    per ISA struct (1 for DMA pseudo-instructions, 2 for compute engines).
    Tile attaches more. Move the excess onto same-engine NoOps inserted just
    before the instruction — NX executes the stream in order, so semantics
    are preserved (walrus accepts many waits on NoOp)."""
    ctr = 0
    for fn in nc.m.functions:
        for blk in fn.blocks:
            insts = blk.instructions
            idx = 0
            while idx < len(insts):
                inst = insts[idx]
                si = inst.sync_info
                if si is None or len(si.on_wait or []) <= 1:
                    idx += 1
                    continue
                waits = list(si.on_wait)
                for w in waits[:-1]:
                    carrier = mybir.InstNoOp(
                        name=f"I-wsplit-{ctr}",
                        sync_info=mybir.SyncInfo(on_wait=[w], on_update=[]),
                        bass_nofuse=True,
                        engine=inst.engine,
                    )
                    ctr += 1
                    blk.instructions.insert(idx, carrier)
                    idx += 1
                si.on_wait = waits[-1:]
                idx += 1


def _emit(nc, tc, w_in, xblk_in, xsum_in, dsum0_in, dsumA_in, drep_in, v_out):
    from contextlib import ExitStack

    ctx = ExitStack()
    with ctx:
        singles = ctx.enter_context(tc.tile_pool(name="singles", bufs=1))
        upool = ctx.enter_context(tc.tile_pool(name="u", bufs=1))
        dpool = ctx.enter_context(tc.tile_pool(name="dram", bufs=1, space="DRAM"))
        pmix = ctx.enter_context(tc.tile_pool(name="pmix", bufs=2, space="PSUM"))
        smalls = ctx.enter_context(tc.tile_pool(name="smalls", bufs=1))
        xbp = ctx.enter_context(tc.tile_pool(name="xb", bufs=1))
        wst = ctx.enter_context(tc.tile_pool(name="wst", bufs=5))
        ppu = ctx.enter_context(tc.tile_pool(name="ppu", bufs=3, space="PSUM"))
        pjk = ctx.enter_context(tc.tile_pool(name="pjk", bufs=1, space="PSUM"))
        m3p = ctx.enter_context(tc.tile_pool(name="m3p", bufs=2))
        m6p = ctx.enter_context(tc.tile_pool(name="m6p", bufs=1))
        trp = ctx.enter_context(tc.tile_pool(name="trp", bufs=1))
        t5p = ctx.enter_context(tc.tile_pool(name="t5p", bufs=1))
        smp = ctx.enter_context(tc.tile_pool(name="smp", bufs=1))

        # ---- resident tiles ----
        u_sb = [
            upool.tile([128, KG, OJ], BF16, tag=f"u{bh}", name=f"u{bh}") for bh in range(2)
        ]
        dsum0_sb = [singles.tile([128, 32], BF16, tag=f"ds0_{b}", name=f"ds0_{b}") for b in range(2)]
        dsumA_sb = [singles.tile([128, 32], BF16, tag=f"dsA_{b}", name=f"dsA_{b}") for b in range(2)]
        drep_sb = [singles.tile([32, 128], BF16, tag=f"drp_{b}", name=f"drp_{b}") for b in range(2)]
        blog = [singles.tile([128, KG, OC], BF16, tag=f"blog{bh}", name=f"blog{bh}") for bh in range(2)]
        # c (normalized) duplicated x2 along a trailing pair axis so the c*u
        # multiply's last AP dim is stride-1/n=2 -> DVE 2x mode
        c3e = [
            singles.tile([128, KG, OC, 2], BF16, tag=f"c3e{bh}", name=f"c3e{bh}")
            for bh in range(2)
        ]
        arep = [singles.tile([128, OJ], BF16, tag=f"arep{bh}", name=f"arep{bh}") for bh in range(2)]

        for bh in range(2):
            nc.sync.dma_start(dsum0_sb[bh], dsum0_in[bh])
            nc.sync.dma_start(dsumA_sb[bh], dsumA_in[bh])
            nc.sync.dma_start(drep_sb[bh], drep_in[bh])
            nc.vector.memset(blog[bh], 0.0)

        # ---- phase 1: produce u = x @ W (per k-group block-diag matmuls),
        # and fuse iteration-0's uniform weighted sum (c = 1/OC, folded into
        # xsum on the host) as two extra matmuls per kg on the same W stream.
        xblk_sb = xbp.tile([128, 2, KG, 128], BF16)
        nc.sync.dma_start(xblk_sb, xblk_in[:])
        xsum_sb = singles.tile([128, KG, 32], BF16, tag="xsum", name="xsum")
        nc.sync.dma_start(xsum_sb, xsum_in[:])
        jxs = singles.tile([1, 128], BF16, tag="jxs", name="jxs")
        nc.sync.dma_start(jxs, w_in[0][0:1, 0:128])
        junk = pjk.tile([2, 8], F32, name="junk")
        s0ps = pmix.tile([B, OJ], F32, tag="pm", name="s0ps")
        # absorb the xsum-DMA and s0ps-slot waits off the accumulation chain
        nc.tensor.matmul(
            s0ps[0:2, 0:2],
            lhsT=xsum_sb[:, 0, 0:2],
            rhs=xsum_sb[:, 0, 0:2],
            start=True,
            stop=True,
        )
        for kg in range(KG):
            w4 = wst.tile([128, OJ], BF16, tag="w4", name="w4")
            nc.gpsimd.dma_start(w4, w_in[kg])
            # tiny matmul reads w4 first so the real matmuls don't carry the
            # DMA wait (MM ISA struct allows only 2 sync waits)
            nc.tensor.matmul(
                junk[:, 0:2],
                lhsT=xblk_sb[:, 0, kg, 0:2],
                rhs=w4[:, 0:2],
                start=True,
                stop=True,
            )
            for bh in range(2):
                for h in range(2):
                    pu = ppu.tile([128, 512], F32, tag="pu", name="pu")
                    nc.tensor.matmul(
                        pu,
                        lhsT=xblk_sb[:, bh, kg, :],
                        rhs=w4[:, 512 * h : 512 * h + 512],
                        start=True,
                        stop=True,
                    )
                    if bh == 0:
                        nc.vector.tensor_copy(
                            out=u_sb[bh][:, kg, 512 * h : 512 * h + 512],
                            in_=pu,
                        )
                    else:
                        nc.scalar.copy(
                            out=u_sb[bh][:, kg, 512 * h : 512 * h + 512],
                            in_=pu,
                        )
            for h in range(2):
                nc.tensor.matmul(
                    s0ps[:, 512 * h : 512 * h + 512],
                    lhsT=xsum_sb[:, kg, :],
                    rhs=w4[:, 512 * h : 512 * h + 512],
                    start=(kg == 0),
                    stop=(kg == KG - 1),
                )

        # ---- helpers ----
        def weighted_sum_pass():
            """s_partial = sum_k (c * u) via DVE mult + delta-ones PE reduce.
            The multiply views the pair-duplicated c3e so its last AP dim is
            stride-1/n=2 (all operands bf16) -> DVE 2x perf mode."""
            ps = pmix.tile([B, OJ], F32, tag="pm", name="ps")
            # absorb the psum-slot-handoff + bank-guard waits so the real
            # accumulation-start matmul stays within the 2-sync-wait MM limit
            nc.tensor.matmul(
                ps[0:2, 0:2],
                lhsT=dsum0_sb[0][:, 0:2],
                rhs=dsum0_sb[0][:, 0:2],
                start=True,
                stop=True,
            )
            for bh in range(2):
                lhs = dsumA_sb[bh]
                for kg in range(KG):
                    m3 = m3p.tile([128, OC, OV], BF16, tag="m3")
                    nc.vector.tensor_mul(
                        out=m3.rearrange("p o (a b) -> p o a b", b=2),
                        in0=u_sb[bh][:, kg].rearrange(
                            "p (o a b) -> p o a b", o=OC, b=2
                        ),
                        in1=c3e[bh][:, kg, :, None, :].to_broadcast(
                            [128, OC, OV // 2, 2]
                        ),
                    )
                    rhs_full = m3.rearrange("p o j -> p (o j)")
                    for h in range(2):
                        nc.tensor.matmul(
                            ps[:, 512 * h : 512 * h + 512],
                            lhsT=lhs,
                            rhs=rhs_full[:, 512 * h : 512 * h + 512],
                            start=(bh == 0 and kg == 0),
                            stop=(bh == 1 and kg == KG - 1),
                        )
            return ps

        def allreduce_s(ps, it):
            s_sb = smalls.tile([B, OJ], F32, tag="s_sb", name="s_sb")
            nc.vector.tensor_copy(out=s_sb, in_=ps)
            sp = dpool.tile([B, OJ], F32, tag=f"sp{it}", name=f"sp{it}")
            sf = dpool.tile([B, OJ], F32, tag=f"sf{it}", name=f"sf{it}")
            nc.gpsimd.dma_start(sp, s_sb)
            nc.gpsimd.collective_compute(
                "AllReduce",
                mybir.AluOpType.add,
                replica_groups=[list(range(NCORES))],
                ins=[sp.opt()],
                outs=[sf.opt()],
            )
            sf_sb = smalls.tile([B, OJ], F32, tag="sf_sb", name="sf_sb")
            nc.gpsimd.dma_start(sf_sb, sf)
            return sf_sb

        def squash(sf_sb, out_dtype, tag):
            sq = smalls.tile([B, OJ], F32, tag="sq", name="sq")
            nc.vector.tensor_mul(out=sq, in0=sf_sb, in1=sf_sb)
            n2 = smalls.tile([B, OC], F32, tag="n2", name="n2")
            nc.vector.reduce_sum(
                n2, sq.rearrange("b (o j) -> b o j", j=OV), axis=mybir.AxisListType.X
            )
            rt = smalls.tile([B, OC], F32, tag="rt", name="rt")
            nc.scalar.activation(rt, n2, mybir.ActivationFunctionType.Sqrt)
            dn = smalls.tile([B, OC], F32, tag="dn", name="dn")
            nc.vector.tensor_scalar_add(dn, n2, 1.0)
            nc.vector.reciprocal(dn, dn)
            f = smalls.tile([B, OC], F32, tag="f", name="f")
            nc.vector.tensor_mul(out=f, in0=rt, in1=dn)
            a_sb = smalls.tile([B, OC, OV], out_dtype, tag="av", name=tag)
            nc.vector.tensor_mul(
                out=a_sb,
                in0=sf_sb.rearrange("b (o j) -> b o j", j=OV),
                in1=f[:, :, None].to_broadcast([B, OC, OV]),
            )
            return a_sb

        def broadcast_a(a_sb):
            for bh in range(2):
                pr = pmix.tile([128, OJ], F32, tag="pm", name="pr")
                nc.tensor.matmul(
                    pr[0:2, 0:2],
                    lhsT=dsum0_sb[0][:, 0:2],
                    rhs=dsum0_sb[0][:, 0:2],
                    start=True,
                    stop=True,
                )
                af = a_sb.rearrange("b o j -> b (o j)")
                for h in range(2):
                    nc.tensor.matmul(
                        pr[:, 512 * h : 512 * h + 512],
                        lhsT=drep_sb[bh],
                        rhs=af[:, 512 * h : 512 * h + 512],
                        start=True,
                        stop=True,
                    )
                nc.scalar.copy(out=arep[bh], in_=pr)

        CH = 4  # kg per B-pass chunk

        def logit_update_pass():
            """blog += sum_j (a * u), via DVE mult + contiguous-halves tree."""
            if True:
                for bh in range(2):
                    for cc in range(KG // CH):
                        k0 = CH * cc
                        m6 = m6p.tile([128, CH, OJ], BF16, tag="m6", name="m6")
                        nc.vector.tensor_mul(
                            out=m6,
                            in0=u_sb[bh][:, k0 : k0 + CH],
                            in1=arep[bh][:, None, :].to_broadcast([128, CH, OJ]),
                        )
                        tt = trp.tile([128, CH, OC, 16], BF16, tag="tt", name="tt")
                        m6v = m6.rearrange("p c (o j) -> p c o j", j=OV)
                        # lvl1 -> tt[...,0:16]; lvl2 -> m6[...,0:8] (consumed);
                        # lvl3 -> tt[...,0:4] (lvl1 consumed); lvl4 -> tt[...,4:6]
                        nc.vector.tensor_add(
                            out=tt, in0=m6v[..., 0:16], in1=m6v[..., 16:32]
                        )
                        nc.vector.tensor_add(
                            out=m6v[..., 0:8], in0=tt[..., 0:8], in1=tt[..., 8:16]
                        )
                        nc.vector.tensor_add(
                            out=tt[..., 0:4], in0=m6v[..., 0:4], in1=m6v[..., 4:8]
                        )
                        nc.vector.tensor_add(
                            out=tt[..., 4:6], in0=tt[..., 0:2], in1=tt[..., 2:4]
                        )
                        t5 = t5p.tile([128, CH, OC], BF16, tag="t5", name="t5")
                        nc.vector.tensor_add(
                            out=t5[:, :, :, None],
                            in0=tt[..., 4:5],
                            in1=tt[..., 5:6],
                        )
                        nc.vector.tensor_add(
                            out=blog[bh][:, k0 : k0 + CH],
                            in0=blog[bh][:, k0 : k0 + CH],
                            in1=t5,
                        )

        def softmax_pass():
            # exp(blog) lands directly in the pair-duplicated c3e; the summed
            # pair-duplicate doubles the denominator, so fold a x2 into the
            # reciprocal before normalizing in place.
            for bh in range(2):
                nc.scalar.activation(
                    c3e[bh],
                    blog[bh][:, :, :, None].to_broadcast([128, KG, OC, 2]),
                    mybir.ActivationFunctionType.Exp,
                )
                den = smp.tile([128, KG], F32, tag="den", name="den")
                nc.vector.reduce_sum(
                    den,
                    c3e[bh].rearrange("p k o b -> p k (o b)"),
                    axis=mybir.AxisListType.X,
                )
                nc.vector.reciprocal(den, den)
                nc.vector.tensor_scalar_mul(den, den, 2.0)
                nc.vector.tensor_mul(
                    out=c3e[bh],
                    in0=c3e[bh],
                    in1=den[:, :, None, None].to_broadcast([128, KG, OC, 2]),
                )

        # ---- routing iterations (it0's s came from the fused phase-1 matmuls)
        for it in range(T):
            ps = s0ps if it == 0 else weighted_sum_pass()
            sf_sb = allreduce_s(ps, it)
            if it < T - 1:
                a_sb = squash(sf_sb, BF16, tag="a_sb")
                broadcast_a(a_sb)
                logit_update_pass()
                softmax_pass()
            else:
                vt = squash(sf_sb, F32, tag="v_sb")
                nc.gpsimd.dma_start(v_out[:], vt.rearrange("b o j -> b (o j)"))


def _host_inputs(x, W):
    """Build per-core staged inputs (numpy, bf16) from full x [B,IC,IV], W [OC,IC,IV,OV]."""
    ins = []
    # constants, identical per core
    dsum0 = np.zeros((2, 128, 32), np.float32)
    dsumA = np.zeros((2, 128, 32), np.float32)
    drep = np.zeros((2, 32, 128), np.float32)
    for bh in range(2):
        for p in range(128):
            bl, k8 = p // 8, p % 8
            dsum0[bh, p, 16 * bh + bl] = 1.0 / OC
            dsumA[bh, p, 16 * bh + bl] = 1.0
            drep[bh, 16 * bh + bl, p] = 1.0
    dsum0 = dsum0.astype(_BF)
    dsumA = dsumA.astype(_BF)
    drep = drep.astype(_BF)

    for c in range(NCORES):
        ksl = slice(KL * c, KL * (c + 1))
        Wc = np.ascontiguousarray(W[:, ksl])  # [o, 256, i, j]
        # -> [kg, (k8 i), (o j)]
        wr = (
            Wc.reshape(OC, KG, 8, IV, OV)
            .transpose(1, 2, 3, 0, 4)
            .reshape(KG, 128, OJ)
            .astype(_BF)
        )
        xc = np.ascontiguousarray(x[:, ksl])  # [32, 256, 16]
        xr = xc.reshape(2, 16, KG, 8, IV)  # [bh, bl, kg, k8, i]
        xb = np.zeros((8, IV, 2, KG, 16, 8), np.float32)  # [k8,i,bh,kg,bl,k8']
        for k8 in range(8):
            xb[k8, :, :, :, :, k8] = xr[:, :, :, k8, :].transpose(3, 0, 2, 1)
        xblk = xb.reshape(128, 2, KG, 128).astype(_BF)
        # xsum[(k8,i), kg, b] = x[b, kg*8+k8, i] / OC  (iteration-0 uniform c)
        xsum = (
            (xr.transpose(3, 4, 2, 0, 1) / OC)  # [k8, i, kg, bh, bl]
            .reshape(128, KG, 32)
            .astype(_BF)
        )
        ins.append(
            {
                "w": wr,
                "xblk": xblk,
                "xsum": xsum,
                "dsum0": dsum0,
                "dsumA": dsumA,
                "drep": drep,
            }
        )
    return ins


def kernel(x: np.ndarray, W: np.ndarray) -> np.ndarray:
    from concourse.bass_utils import run_bass_kernel_spmd

    x = np.asarray(x, np.float32)
    W = np.asarray(W, np.float32)
    nc = build_nc()
    in_maps = _host_inputs(x, W)
    res = run_bass_kernel_spmd(nc, in_maps, core_ids=list(range(NCORES)))
    v = res.results[0]["v"].reshape(B, OC, OV).astype(np.float32)
    return v


if __name__ == "__main__":
    rng = np.random.default_rng(0)
    x = rng.standard_normal((B, IC, IV), dtype=np.float32)
    W = (0.01 * rng.standard_normal((OC, IC, IV, OV))).astype(np.float32)
    v = kernel(x, W)
    print("v", v.shape, v.dtype, float(np.abs(v).max()))



# revision 2
# speedup vs baseline: 1.2501x; 1.2501x over previous
"""Capsule-routing kernel for Trainium2 (8 NeuronCores, Bass/Tile) — v2.

Problem: u[b,o,k,j] = sum_i x[b,k,i] W[o,k,i,j]; 3 iters of dynamic routing
(softmax over o of per-(b,k) logits, squash over j), output v[b,o,j].

Sharding: input-capsule (IC=2048) dim split across 8 cores (256 each).

v2 structure (vs the original):
- Phase 1 streams W through TWO DMA rings (SP + Pool queues): ring A feeds
  the s0 accumulation matmuls (iteration-0 weighted sum, uniform c folded
  into xsum on the host) which chase the DMA and finish early, so the s0
  AllReduce overlaps the remaining u-production matmuls + PSUM drains.
- PSUM drains of u split DVE (bh0) / ACT (bh1), one [128,1024] copy per
  (kg, bh).
- Routing iterations are fused per (bh, cc-chunk): m6 = a*u -> add-tree ->
  blog += -> exp -> den -> normalize -> m3 = c*u -> PE delta-reduce, with a
  static engine assignment balancing DVE vs Pool.
- s lives as [128, 256] on-chip (full partition utilization for squash);
  sqrt is computed as exp(0.5*ln(.)) so ACT stays on one activation table;
  a is broadcast to the u layout by DMA via a DRAM bounce, not PE matmuls.
- Final collective is a ReduceScatter (cheaper than AllReduce); each core
  squashes its 4-sample slice and the host concatenates the 8 slices.

Shapes (hardcoded): B=32, IC=2048, IV=16, OC=32, OV=32, T=3.
"""

import sys

sys.path.insert(0, "/opt/trn_rl_repo")

import numpy as np
import ml_dtypes

import concourse.bass as bass
import concourse.mybir as mybir
import concourse.tile as tile

BF16 = mybir.dt.bfloat16
FP16 = mybir.dt.float16
F32 = mybir.dt.float32

NCORES = 8
B, IC, IV, OC, OV = 32, 2048, 16, 32, 32
KL = IC // NCORES          # 256 local input capsules per core
KG = KL // 8               # 32 k-groups of 8
OJ = OC * OV               # 1024
T = 3
CH = 4                     # kg per fused-routing chunk
NCC = KG // CH             # 8 chunks per bh

_BF = ml_dtypes.bfloat16


def build_nc(split_waits=True):
    nc = bass.Bass()

    wa_in = nc.declare_dram_parameter("wa", [KG, 128, OJ], BF16, isOutput=False)
    wb_in = nc.declare_dram_parameter("wb", [KG, 128, OJ], BF16, isOutput=False)
    xblk_in = nc.declare_dram_parameter("xblk", [128, 2, KG, 128], BF16, isOutput=False)
    xsum_in = nc.declare_dram_parameter("xsum", [128, KG, 32], BF16, isOutput=False)
    dsum0_in = nc.declare_dram_parameter("dsum0", [2, 128, 32], BF16, isOutput=False)
    dsumA_in = nc.declare_dram_parameter("dsumA", [2, 128, 32], BF16, isOutput=False)
    v_out = nc.declare_dram_parameter("v", [B // NCORES, OJ], F32, isOutput=True)

    with tile.TileContext(nc) as tc:
        with nc.allow_low_precision(reason="bf16 routing intermediates"):
            _emit(nc, tc, wa_in, wb_in, xblk_in, xsum_in, dsum0_in, dsumA_in, v_out)
    if split_waits:
        _split_excess_waits(nc)
    return nc


def _split_excess_waits(nc):
    """The walrus build in this container accepts very few sync-wait commands
    per ISA struct (1 for DMA pseudo-instructions, 2 for compute engines).
    Tile attaches more. Move the excess onto same-engine NoOps inserted just
    before the instruction — NX executes the stream in order, so semantics
    are preserved (walrus accepts many waits on NoOp)."""
    ctr = 0
    for fn in nc.m.functions:
        for blk in fn.blocks:
            insts = blk.instructions
            idx = 0
            while idx < len(insts):
                inst = insts[idx]
                si = inst.sync_info
                if si is None or len(si.on_wait or []) <= 1:
                    idx += 1
                    continue
                waits = list(si.on_wait)
                for w in waits[:-1]:
                    carrier = mybir.InstNoOp(
                        name=f"I-wsplit-{ctr}",
                        sync_info=mybir.SyncInfo(on_wait=[w], on_update=[]),
                        bass_nofuse=True,
                        engine=inst.engine,
                    )
                    ctr += 1
                    blk.instructions.insert(idx, carrier)
                    idx += 1
                si.on_wait = waits[-1:]
                idx += 1


def _emit(nc, tc, wa_in, wb_in, xblk_in, xsum_in, dsum0_in, dsumA_in, v_out):
    from contextlib import ExitStack

    ctx = ExitStack()
    with ctx:
        singles = ctx.enter_context(tc.tile_pool(name="singles", bufs=1))
        upool = ctx.enter_context(tc.tile_pool(name="u", bufs=1))
        dpool = ctx.enter_context(tc.tile_pool(name="dram", bufs=1, space="DRAM"))
        pmix = ctx.enter_context(tc.tile_pool(name="pmix", bufs=1, space="PSUM"))
        smalls = ctx.enter_context(tc.tile_pool(name="smalls", bufs=1))
        xbp = ctx.enter_context(tc.tile_pool(name="xb", bufs=1))
        wsa = ctx.enter_context(tc.tile_pool(name="wsa", bufs=3))
        wsb = ctx.enter_context(tc.tile_pool(name="wsb", bufs=2))
        ppu = ctx.enter_context(tc.tile_pool(name="ppu", bufs=2, space="PSUM"))
        pjk = ctx.enter_context(tc.tile_pool(name="pjk", bufs=1, space="PSUM"))
        mprod = ctx.enter_context(tc.tile_pool(name="mprod", bufs=3))
        trp = ctx.enter_context(tc.tile_pool(name="trp", bufs=2))
        t5p = ctx.enter_context(tc.tile_pool(name="t5p", bufs=2))
        c3p = ctx.enter_context(tc.tile_pool(name="c3p", bufs=3))
        smp = ctx.enter_context(tc.tile_pool(name="smp", bufs=2))

        # ---- resident tiles ----
        u_sb = [
            upool.tile([128, KG, OJ], BF16, tag=f"u{bh}", name=f"u{bh}") for bh in range(2)
        ]
        dsum0_sb = [singles.tile([128, 32], BF16, tag=f"ds0_{b}", name=f"ds0_{b}") for b in range(2)]
        dsumA_sb = [singles.tile([128, 32], BF16, tag=f"dsA_{b}", name=f"dsA_{b}") for b in range(2)]
        blog = [singles.tile([128, KG, OC], BF16, tag=f"blog{bh}", name=f"blog{bh}") for bh in range(2)]
        arep2 = singles.tile([128, 2, OJ], BF16, tag="arep", name="arep")

        for bh in range(2):
            nc.sync.dma_start(dsum0_sb[bh], dsum0_in[bh])
            nc.sync.dma_start(dsumA_sb[bh], dsumA_in[bh])
            nc.vector.memset(blog[bh], 0.0)
        # preload the ln/exp activation table off the critical path (squash
        # computes sqrt as exp(0.5*ln) and softmax uses exp; one resident
        # table serves ln, exp and copy)
        lnwarm = smalls.tile([1, 2], F32, tag="lnwarm", name="lnwarm")
        nc.vector.memset(lnwarm, 1.0)
        nc.scalar.activation(lnwarm, lnwarm, mybir.ActivationFunctionType.Ln)

        # ---- phase 1 ----
        # Ring A (SP queue) feeds the s0 matmuls; ring B (Pool queue) feeds
        # u-production. s0 completes ~when ring A drains, so its AllReduce
        # overlaps the tail of u-production.
        xblk_sb = xbp.tile([128, 2, KG, 128], BF16)
        nc.scalar.dma_start(xblk_sb, xblk_in[:])
        xsum_sb = singles.tile([128, KG, 32], BF16, tag="xsum", name="xsum")
        nc.scalar.dma_start(xsum_sb, xsum_in[:])
        junk = pjk.tile([2, 8], F32, name="junk")
        s0ps = pmix.tile([B, OJ], F32, tag="pm", name="s0ps")
        # absorb the xsum-DMA and s0ps-slot waits off the accumulation chain
        nc.tensor.matmul(
            s0ps[0:2, 0:2],
            lhsT=xsum_sb[:, 0, 0:2],
            rhs=xsum_sb[:, 0, 0:2],
            start=True,
            stop=True,
        )
        # s0 chain (ring A)
        for kg in range(KG):
            w4 = wsa.tile([128, OJ], BF16, tag="wa", name="wa")
            nc.sync.dma_start(w4, wa_in[kg])
            for h in range(2):
                nc.tensor.matmul(
                    s0ps[:, 512 * h : 512 * h + 512],
                    lhsT=xsum_sb[:, kg, :],
                    rhs=w4[:, 512 * h : 512 * h + 512],
                    start=(kg == 0),
                    stop=(kg == KG - 1),
                )
        # u chain (ring B) — emitted by the caller after R0 is enqueued
        def emit_ring_b():
            for kg in range(KG):
                w4 = wsb.tile([128, OJ], BF16, tag="wb", name="wb")
                with tc.tile_wait_until(0.0345):
                    nc.sync.dma_start(w4, wb_in[kg])
                nc.tensor.matmul(
                    junk[:, 2:4],
                    lhsT=xblk_sb[:, 0, kg, 0:2],
                    rhs=w4[:, 0:2],
                    start=True,
                    stop=True,
                )
                for bh in range(2):
                    pu = ppu.tile([128, OJ], F32, tag="pu", name="pu")
                    for h in range(2):
                        nc.tensor.matmul(
                            pu[:, 512 * h : 512 * h + 512],
                            lhsT=xblk_sb[:, bh, kg, :],
                            rhs=w4[:, 512 * h : 512 * h + 512],
                            start=True,
                            stop=True,
                        )
                    if bh == 0:
                        nc.vector.tensor_copy(out=u_sb[bh][:, kg], in_=pu)
                    else:
                        nc.scalar.copy(out=u_sb[bh][:, kg], in_=pu)

        # ---- helpers ----
        def allreduce_s(ps, it):
            """AllReduce the [B, OJ] psum accumulator; return SBUF [128, 256]
            view (partition = (b, og4), free = (o8, j))."""
            s_sb = smalls.tile([B, OJ], F32, tag="s_sb", name=f"s_sb{it}")
            nc.vector.tensor_copy(out=s_sb, in_=ps)
            sp = dpool.tile([B, OJ], F32, tag=f"sp{it}", name=f"sp{it}")
            sf = dpool.tile([B, OJ], F32, tag=f"sf{it}", name=f"sf{it}")
            nc.gpsimd.dma_start(sp, s_sb)
            nc.gpsimd.collective_compute(
                "AllReduce",
                mybir.AluOpType.add,
                replica_groups=[list(range(NCORES))],
                ins=[sp.opt()],
                outs=[sf.opt()],
            )
            sf_sb = smalls.tile([128, 256], F32, tag="sf_sb", name=f"sf_sb{it}")
            nc.scalar.dma_start(sf_sb, sf.rearrange("b (og r) -> (b og) r", og=4))
            return sf_sb

        def ln_sqrt(rt, n2, tag):
            """rt = sqrt(n2) via exp(0.5*ln n2) — keeps ACT on the ln/exp
            activation table (no table reload between squash and softmax)."""
            p = n2.shape[0]
            lnn = smalls.tile(list(n2.shape), F32, tag="lnn", name=f"lnn{tag}")
            nc.scalar.activation(lnn, n2, mybir.ActivationFunctionType.Ln)
            nc.scalar.activation(rt, lnn, mybir.ActivationFunctionType.Exp, 0.0, 0.5)
            return rt

        def squash_wide(sf_sb, it):
            """squash on [128=(b,og), 256=(o8,j)] -> a (bf16) staged to DRAM."""
            s3 = sf_sb.rearrange("p (o j) -> p o j", j=OV)
            sq = smalls.tile([128, 8, OV], F32, tag="sq", name=f"sq{it}")
            nc.vector.tensor_mul(out=sq, in0=s3, in1=s3)
            n2 = smalls.tile([128, 8], F32, tag="n2", name=f"n2{it}")
            nc.vector.reduce_sum(n2, sq, axis=mybir.AxisListType.X)
            rt = smalls.tile([128, 8], F32, tag="rt", name=f"rt{it}")
            ln_sqrt(rt, n2, f"s{it}")
            dn = smalls.tile([128, 8], F32, tag="dn", name=f"dn{it}")
            nc.vector.tensor_scalar_add(dn, n2, 1.0)
            nc.vector.reciprocal(dn, dn)
            f = smalls.tile([128, 8], F32, tag="f", name=f"f{it}")
            nc.vector.tensor_mul(out=f, in0=rt, in1=dn)
            a_sb = smalls.tile([128, 8, OV], BF16, tag="aw", name=f"aw{it}")
            nc.vector.tensor_mul(
                out=a_sb,
                in0=s3,
                in1=f[:, :, None].to_broadcast([128, 8, OV]),
            )
            a_dr = dpool.tile([128, 256], BF16, tag=f"adr{it}", name=f"adr{it}")
            nc.sync.dma_start(a_dr, a_sb.rearrange("p o j -> p (o j)"))
            return a_dr

        def broadcast_a(a_dr):
            """arep2[(bl,k8), bh, (o,j)] = a[b=bh*16+bl, (o,j)], replicated
            over k8: one [16, 2, 1024] DMA per k8 (partition stride 8), spread
            over four queues. Row b of a_dr's (b,og)-major layout is the
            contiguous 2048B (o,j) row."""
            av = a_dr.rearrange("(h bl og) r -> bl h (og r)", h=2, og=4)
            dst = arep2.rearrange("(k8 bl) h f -> k8 bl h f", k8=8)
            qs = [nc.sync, nc.scalar, nc.gpsimd]
            for k8 in range(8):
                qs[k8 % 3].dma_start(dst[k8], av)

        def fused_iteration(it):
            """lu(it) + softmax -> c(it+1) + ws(it+1), software-pipelined:
            stage A(i) = m6/tree/blog/exp/den-tree of chunk i;
            stage B1(i) = recip/den2 of chunk i; B2(i) = norm/m3/ws of i.
            Emission per index i: A(i), B1(i-1), B2(i-2) — so neither DVE nor
            Pool ever waits on the intra-chunk chain."""
            ps = pmix.tile([B, OJ], F32, tag="pm", name=f"ps{it}")
            nc.tensor.matmul(
                ps[0:2, 0:2],
                lhsT=dsum0_sb[0][:, 0:2],
                rhs=dsum0_sb[0][:, 0:2],
                start=True,
                stop=True,
            )
            NCH = 2 * NCC
            state = {}

            def stage_a(i):
                bh, cc = divmod(i, NCC)
                k0 = CH * cc
                u_c = u_sb[bh][:, k0 : k0 + CH]
                m6 = mprod.tile([128, CH, OJ], BF16, tag="prod", name="m6")
                nc.vector.tensor_mul(
                    out=m6,
                    in0=u_c,
                    in1=arep2[:, bh, None, :].to_broadcast([128, CH, OJ]),
                )
                m6v = m6.rearrange("p c (o j) -> p c o j", j=OV)
                tt = trp.tile([128, CH, OC, 16], BF16, tag="tt", name="tt")
                nc.gpsimd.tensor_add(
                    out=tt, in0=m6v[..., 0:16], in1=m6v[..., 16:32]
                )
                nc.gpsimd.tensor_add(
                    out=tt[..., 0:8], in0=tt[..., 0:8], in1=tt[..., 8:16]
                )
                nc.gpsimd.tensor_add(
                    out=tt[..., 8:12], in0=tt[..., 0:4], in1=tt[..., 4:8]
                )
                nc.gpsimd.tensor_add(
                    out=tt[..., 12:14], in0=tt[..., 8:10], in1=tt[..., 10:12]
                )
                t5 = t5p.tile([128, CH, OC], BF16, tag="t5", name="t5")
                nc.gpsimd.tensor_add(
                    out=t5[:, :, :, None], in0=tt[..., 12:13], in1=tt[..., 13:14]
                )
                nc.gpsimd.tensor_add(
                    out=blog[bh][:, k0 : k0 + CH],
                    in0=blog[bh][:, k0 : k0 + CH],
                    in1=t5,
                )
                c3 = c3p.tile([128, CH, OC], BF16, tag="c3", name="c3")
                nc.scalar.activation(
                    c3,
                    blog[bh][:, k0 : k0 + CH],
                    mybir.ActivationFunctionType.Exp,
                )
                state[i] = (c3, None)  # den-tree deferred to b1

            def stage_b1(i):
                c3, _ = state.pop(i)
                dA = smp.tile([128, CH, 16], FP16, tag="dA", name="dA")
                nc.gpsimd.tensor_add(out=dA, in0=c3[..., 0:16], in1=c3[..., 16:32])
                nc.gpsimd.tensor_add(out=dA[..., 0:8], in0=dA[..., 0:8], in1=dA[..., 8:16])
                nc.gpsimd.tensor_add(out=dA[..., 8:12], in0=dA[..., 0:4], in1=dA[..., 4:8])
                nc.gpsimd.tensor_add(out=dA[..., 12:14], in0=dA[..., 8:10], in1=dA[..., 10:12])
                den = smp.tile([128, CH], F32, tag="den", name="den")
                nc.gpsimd.tensor_add(
                    out=den[:, :, None], in0=dA[..., 12:13], in1=dA[..., 13:14]
                )
                nc.vector.reciprocal(den, den)
                den2 = smp.tile([128, CH, 2], BF16, tag="den2", name="den2")
                nc.vector.tensor_copy(
                    out=den2, in_=den[:, :, None].to_broadcast([128, CH, 2])
                )
                state[i] = (c3, den2)

            def stage_b2_norm(i):
                c3, den2 = state[i]
                c3e = c3p.tile([128, CH, OC, 2], BF16, tag="c3e", name="c3e")
                nc.gpsimd.tensor_mul(
                    out=c3e,
                    in0=c3[:, :, :, None].to_broadcast([128, CH, OC, 2]),
                    in1=den2[:, :, None, :].to_broadcast([128, CH, OC, 2]),
                )
                state[i] = (c3e, den2)

            def stage_b2(i):
                bh, cc = divmod(i, NCC)
                k0 = CH * cc
                u_c = u_sb[bh][:, k0 : k0 + CH]
                c3e, den2 = state.pop(i)
                del den2
                m3 = mprod.tile([128, CH, OC, OV], BF16, tag="prod", name="m3")
                nc.vector.tensor_mul(
                    out=m3.rearrange("p c o (a b) -> p c o a b", b=2),
                    in0=u_c.rearrange("p c (o a b) -> p c o a b", o=OC, b=2),
                    in1=c3e[:, :, :, None, :].to_broadcast(
                        [128, CH, OC, OV // 2, 2]
                    ),
                )
                m3f = m3.rearrange("p c o j -> p c (o j)")
                for ci in range(CH):
                    for h in range(2):
                        nc.tensor.matmul(
                            ps[:, 512 * h : 512 * h + 512],
                            lhsT=dsumA_sb[bh],
                            rhs=m3f[:, ci, 512 * h : 512 * h + 512],
                            start=(i == 0 and ci == 0),
                            stop=(i == NCH - 1 and ci == CH - 1),
                        )

            for i in range(NCH + 2):
                if i >= 2:
                    stage_b2_norm(i - 2)
                if i < NCH:
                    stage_a(i)
                if 1 <= i <= NCH:
                    stage_b1(i - 1)
                if i >= 2:
                    stage_b2(i - 2)
            return ps

        # ---- routing ----
        sf0 = allreduce_s(s0ps, 0)
        emit_ring_b()
        a0 = squash_wide(sf0, 0)
        broadcast_a(a0)
        ps1 = fused_iteration(0)
        sf1 = allreduce_s(ps1, 1)
        a1 = squash_wide(sf1, 1)
        broadcast_a(a1)
        ps2 = fused_iteration(1)

        # ---- final: ReduceScatter s2, squash the local 4-sample slice ----
        s2_sb = smalls.tile([B, OJ], F32, tag="s_sb", name="s2_sb")
        nc.vector.tensor_copy(out=s2_sb, in_=ps2)
        sp2 = dpool.tile([B, OJ], F32, tag="sp2", name="sp2")
        sf2 = dpool.tile([B // NCORES, OJ], F32, tag="sf2", name="sf2")
        nc.gpsimd.dma_start(sp2, s2_sb)
        nc.gpsimd.collective_compute(
            "ReduceScatter",
            mybir.AluOpType.add,
            replica_groups=[list(range(NCORES))],
            ins=[sp2.opt()],
            outs=[sf2.opt()],
        )
        # [4, 1024] -> [32=(b,og8), 128=(o4,j)]
        sfv = smalls.tile([32, 128], F32, tag="sfv", name="sfv")
        nc.scalar.dma_start(sfv, sf2.rearrange("b (og r) -> (b og) r", og=8))
        s3 = sfv.rearrange("p (o j) -> p o j", j=OV)
        sq = smalls.tile([32, 4, OV], F32, tag="sqv", name="sqv")
        nc.vector.tensor_mul(out=sq, in0=s3, in1=s3)
        n2 = smalls.tile([32, 4], F32, tag="n2v", name="n2v")
        nc.vector.reduce_sum(n2, sq, axis=mybir.AxisListType.X)
        rt = smalls.tile([32, 4], F32, tag="rtv", name="rtv")
        ln_sqrt(rt, n2, "v")
        dn = smalls.tile([32, 4], F32, tag="dnv", name="dnv")
        nc.vector.tensor_scalar_add(dn, n2, 1.0)
        nc.vector.reciprocal(dn, dn)
        f = smalls.tile([32, 4], F32, tag="fv", name="fv")
        nc.vector.tensor_mul(out=f, in0=rt, in1=dn)
        vt = smalls.tile([32, 4, OV], F32, tag="vv", name="vv")
        nc.vector.tensor_mul(
            out=vt, in0=s3, in1=f[:, :, None].to_broadcast([32, 4, OV])
        )
        nc.gpsimd.dma_start(
            v_out.rearrange("b (og r) -> (b og) r", og=8),
            vt.rearrange("p o j -> p (o j)"),
        )


def _host_inputs(x, W):
    """Build per-core staged inputs (numpy, bf16) from full x [B,IC,IV], W [OC,IC,IV,OV]."""
    ins = []
    dsum0 = np.zeros((2, 128, 32), np.float32)
    dsumA = np.zeros((2, 128, 32), np.float32)
    for bh in range(2):
        for p in range(128):
            k8, bl = p // 16, p % 16
            dsum0[bh, p, 16 * bh + bl] = 1.0 / OC
            dsumA[bh, p, 16 * bh + bl] = 1.0
    dsum0 = dsum0.astype(_BF)
    dsumA = dsumA.astype(_BF)

    for c in range(NCORES):
        ksl = slice(KL * c, KL * (c + 1))
        Wc = np.ascontiguousarray(W[:, ksl])  # [o, 256, i, j]
        wr = (
            Wc.reshape(OC, KG, 8, IV, OV)
            .transpose(1, 2, 3, 0, 4)
            .reshape(KG, 128, OJ)
            .astype(_BF)
        )
        xc = np.ascontiguousarray(x[:, ksl])  # [32, 256, 16]
        xr = xc.reshape(2, 16, KG, 8, IV)  # [bh, bl, kg, k8, i]
        xb = np.zeros((8, IV, 2, KG, 8, 16), np.float32)  # [k8,i,bh,kg,k8',bl]
        for k8 in range(8):
            xb[k8, :, :, :, k8, :] = xr[:, :, :, k8, :].transpose(3, 0, 2, 1)
        xblk = xb.reshape(128, 2, KG, 128).astype(_BF)
        xsum = (
            (xr.transpose(3, 4, 2, 0, 1) / OC)  # [k8, i, kg, bh, bl]
            .reshape(128, KG, 32)
            .astype(_BF)
        )
        ins.append(
            {
                "wa": wr,
                "wb": wr,
                "xblk": xblk,
                "xsum": xsum,
                "dsum0": dsum0,
                "dsumA": dsumA,
            }
        )
    return ins


def kernel(x: np.ndarray, W: np.ndarray) -> np.ndarray:
    from concourse.bass_utils import run_bass_kernel_spmd

    x = np.asarray(x, np.float32)
    W = np.asarray(W, np.float32)
    nc = build_nc()
    in_maps = _host_inputs(x, W)
    res = run_bass_kernel_spmd(nc, in_maps, core_ids=list(range(NCORES)))
    parts = [
        res.results[c]["v"].reshape(B // NCORES, OC, OV).astype(np.float32)
        for c in range(NCORES)
    ]
    return np.concatenate(parts, axis=0)


if __name__ == "__main__":
    rng = np.random.default_rng(0)
    x = rng.standard_normal((B, IC, IV), dtype=np.float32)
    W = (0.01 * rng.standard_normal((OC, IC, IV, OV))).astype(np.float32)
    v = kernel(x, W)
    print("v", v.shape, v.dtype, float(np.abs(v).max()))


# revision 3
# speedup vs baseline: 1.4080x; 1.1263x over previous
"""Capsule-routing kernel for Trainium2 (8 NeuronCores, Bass/Tile) — v2.

Problem: u[b,o,k,j] = sum_i x[b,k,i] W[o,k,i,j]; 3 iters of dynamic routing
(softmax over o of per-(b,k) logits, squash over j), output v[b,o,j].

Sharding: input-capsule (IC=2048) dim split across 8 cores (256 each).

v2 structure (vs the original):
- Phase 1 streams W through TWO DMA rings (SP + Pool queues): ring A feeds
  the s0 accumulation matmuls (iteration-0 weighted sum, uniform c folded
  into xsum on the host) which chase the DMA and finish early, so the s0
  AllReduce overlaps the remaining u-production matmuls + PSUM drains.
- PSUM drains of u split DVE (bh0) / ACT (bh1), one [128,1024] copy per
  (kg, bh).
- Routing iterations are fused per (bh, cc-chunk): m6 = a*u -> add-tree ->
  blog += -> exp -> den -> normalize -> m3 = c*u -> PE delta-reduce, with a
  static engine assignment balancing DVE vs Pool.
- s lives as [128, 256] on-chip (full partition utilization for squash);
  sqrt is computed as exp(0.5*ln(.)) so ACT stays on one activation table;
  a is broadcast to the u layout by DMA via a DRAM bounce, not PE matmuls.
- Final collective is a ReduceScatter (cheaper than AllReduce); each core
  squashes its 4-sample slice and the host concatenates the 8 slices.

Shapes (hardcoded): B=32, IC=2048, IV=16, OC=32, OV=32, T=3.
"""

import sys

sys.path.insert(0, "/opt/trn_rl_repo")

import numpy as np
import ml_dtypes

import concourse.bass as bass
import concourse.mybir as mybir
import concourse.tile as tile

BF16 = mybir.dt.bfloat16
FP16 = mybir.dt.float16
F32 = mybir.dt.float32

NCORES = 8
B, IC, IV, OC, OV = 32, 2048, 16, 32, 32
KL = IC // NCORES          # 256 local input capsules per core
KG = KL // 8               # 32 k-groups of 8
OJ = OC * OV               # 1024
T = 3
CH = 4                     # kg per fused-routing chunk
NCC = KG // CH             # 8 chunks per bh

_BF = ml_dtypes.bfloat16


def build_nc(split_waits=True):
    nc = bass.Bass()

    wa_in = nc.declare_dram_parameter("wa", [KG, 128, OJ], BF16, isOutput=False)
    wb_in = nc.declare_dram_parameter("wb", [KG, 128, OJ], BF16, isOutput=False)
    xblk_in = nc.declare_dram_parameter("xblk", [128, 2, KG, 128], BF16, isOutput=False)
    xsum_in = nc.declare_dram_parameter("xsum", [128, KG, 32], BF16, isOutput=False)
    dsum0_in = nc.declare_dram_parameter("dsum0", [2, 128, 32], BF16, isOutput=False)
    dsumA_in = nc.declare_dram_parameter("dsumA", [2, 128, 32], BF16, isOutput=False)
    v_out = nc.declare_dram_parameter("v", [B // NCORES, OJ], F32, isOutput=True)

    with tile.TileContext(nc) as tc:
        with nc.allow_low_precision(reason="bf16 routing intermediates"):
            _emit(nc, tc, wa_in, wb_in, xblk_in, xsum_in, dsum0_in, dsumA_in, v_out)
    if split_waits:
        _split_excess_waits(nc)
    return nc


def _split_excess_waits(nc):
    """The walrus build in this container accepts very few sync-wait commands
    per ISA struct (1 for DMA pseudo-instructions, 2 for compute engines).
    Tile attaches more. Move the excess onto same-engine NoOps inserted just
    before the instruction — NX executes the stream in order, so semantics
    are preserved (walrus accepts many waits on NoOp)."""
    ctr = 0
    for fn in nc.m.functions:
        for blk in fn.blocks:
            insts = blk.instructions
            idx = 0
            while idx < len(insts):
                inst = insts[idx]
                si = inst.sync_info
                if si is None or len(si.on_wait or []) <= 1:
                    idx += 1
                    continue
                waits = list(si.on_wait)
                for w in waits[:-1]:
                    carrier = mybir.InstNoOp(
                        name=f"I-wsplit-{ctr}",
                        sync_info=mybir.SyncInfo(on_wait=[w], on_update=[]),
                        bass_nofuse=True,
                        engine=inst.engine,
                    )
                    ctr += 1
                    blk.instructions.insert(idx, carrier)
                    idx += 1
                si.on_wait = waits[-1:]
                idx += 1


def _emit(nc, tc, wa_in, wb_in, xblk_in, xsum_in, dsum0_in, dsumA_in, v_out):
    from contextlib import ExitStack

    ctx = ExitStack()
    with ctx:
        singles = ctx.enter_context(tc.tile_pool(name="singles", bufs=1))
        upool = ctx.enter_context(tc.tile_pool(name="u", bufs=1))
        dpool = ctx.enter_context(tc.tile_pool(name="dram", bufs=1, space="DRAM"))
        pmix = ctx.enter_context(tc.tile_pool(name="pmix", bufs=1, space="PSUM"))
        smalls = ctx.enter_context(tc.tile_pool(name="smalls", bufs=1))
        xbp = ctx.enter_context(tc.tile_pool(name="xb", bufs=1))
        wsa = ctx.enter_context(tc.tile_pool(name="wsa", bufs=4))
        wsb = ctx.enter_context(tc.tile_pool(name="wsb", bufs=2))
        ppu = ctx.enter_context(tc.tile_pool(name="ppu", bufs=2, space="PSUM"))
        pjk = ctx.enter_context(tc.tile_pool(name="pjk", bufs=1, space="PSUM"))
        mprod = ctx.enter_context(tc.tile_pool(name="mprod", bufs=3))
        trp = ctx.enter_context(tc.tile_pool(name="trp", bufs=2))
        t5p = ctx.enter_context(tc.tile_pool(name="t5p", bufs=1))
        c3p = ctx.enter_context(tc.tile_pool(name="c3p", bufs=3))
        cep = ctx.enter_context(tc.tile_pool(name="cep", bufs=2))
        smp = ctx.enter_context(tc.tile_pool(name="smp", bufs=2))

        # ---- resident tiles ----
        u_sb = [
            upool.tile([128, KG, OJ], BF16, tag=f"u{bh}", name=f"u{bh}") for bh in range(2)
        ]
        dsumA_sb = [singles.tile([128, 32], BF16, tag=f"dsA_{b}", name=f"dsA_{b}") for b in range(2)]
        blog = [singles.tile([128, KG, OC], BF16, tag=f"blog{bh}", name=f"blog{bh}") for bh in range(2)]
        arep2 = singles.tile([128, 2, OJ], BF16, tag="arep", name="arep")

        for bh in range(2):
            nc.sync.dma_start(dsumA_sb[bh], dsumA_in[bh])
            nc.vector.memset(blog[bh], 0.0)
        # preload the ln/exp activation table off the critical path (squash
        # computes sqrt as exp(0.5*ln) and softmax uses exp; one resident
        # table serves ln, exp and copy)
        lnwarm = smalls.tile([1, 2], F32, tag="lnn", name="lnwarm")
        nc.vector.memset(lnwarm, 1.0)
        nc.scalar.activation(lnwarm, lnwarm, mybir.ActivationFunctionType.Ln)

        # ---- phase 1 ----
        # Ring A (SP queue) feeds the s0 matmuls; ring B (Pool queue) feeds
        # u-production. s0 completes ~when ring A drains, so its AllReduce
        # overlaps the tail of u-production.
        xblk_sb = xbp.tile([128, 2, KG, 128], BF16)
        nc.scalar.dma_start(xblk_sb, xblk_in[:])
        xsum_sb = singles.tile([128, KG, 32], BF16, tag="xsum", name="xsum")
        nc.scalar.dma_start(xsum_sb, xsum_in[:])
        junk = pjk.tile([2, 8], F32, name="junk")
        s0ps = pmix.tile([B, OJ], F32, tag="pm", name="s0ps")
        # absorb the xsum-DMA and s0ps-slot waits off the accumulation chain
        nc.tensor.matmul(
            s0ps[0:2, 0:2],
            lhsT=xsum_sb[:, 0, 0:2],
            rhs=xsum_sb[:, 0, 0:2],
            start=True,
            stop=True,
        )
        # s0 chain (ring A)
        for kg in range(KG):
            w4 = wsa.tile([128, OJ], BF16, tag="wa", name="wa")
            nc.sync.dma_start(w4, wa_in[kg])
            for h in range(2):
                nc.tensor.matmul(
                    s0ps[:, 512 * h : 512 * h + 512],
                    lhsT=xsum_sb[:, kg, :],
                    rhs=w4[:, 512 * h : 512 * h + 512],
                    start=(kg == 0),
                    stop=(kg == KG - 1),
                )
        # u chain (ring B) — emitted by the caller after R0 is enqueued
        def emit_ring_b():
            for kg in range(KG):
                w4 = wsb.tile([128, OJ], BF16, tag="wb", name="wb")
                with tc.tile_wait_until(0.0345):
                    nc.sync.dma_start(w4, wb_in[kg])
                nc.tensor.matmul(
                    junk[:, 2:4],
                    lhsT=xblk_sb[:, 0, kg, 0:2],
                    rhs=w4[:, 0:2],
                    start=True,
                    stop=True,
                )
                for bh in range(2):
                    pu = ppu.tile([128, OJ], F32, tag="pu", name="pu")
                    for h in range(2):
                        nc.tensor.matmul(
                            pu[:, 512 * h : 512 * h + 512],
                            lhsT=xblk_sb[:, bh, kg, :],
                            rhs=w4[:, 512 * h : 512 * h + 512],
                            start=True,
                            stop=True,
                        )
                    if bh == 0:
                        nc.vector.tensor_copy(out=u_sb[bh][:, kg], in_=pu)
                    else:
                        nc.scalar.copy(out=u_sb[bh][:, kg], in_=pu)

        # ---- helpers ----
        def allreduce_s(ps, it):
            """AllReduce the [B, OJ] psum accumulator; return SBUF [128, 256]
            view (partition = (b, og4), free = (o8, j))."""
            s_sb = smalls.tile([B, OJ], BF16, tag="s_sb", name=f"s_sb{it}")
            nc.vector.tensor_copy(out=s_sb, in_=ps)
            sp = dpool.tile([B, OJ], BF16, tag=f"sp{it}", name=f"sp{it}")
            sf = dpool.tile([B, OJ], BF16, tag=f"sf{it}", name=f"sf{it}")
            nc.gpsimd.dma_start(sp, s_sb)
            nc.gpsimd.collective_compute(
                "AllReduce",
                mybir.AluOpType.add,
                replica_groups=[list(range(NCORES))],
                ins=[sp.opt()],
                outs=[sf.opt()],
            )
            sf_sb = smalls.tile([128, 256], BF16, tag="sf_sb", name=f"sf_sb{it}")
            nc.sync.dma_start(sf_sb, sf.rearrange("b (og r) -> (b og) r", og=4))
            return sf_sb

        def ln_sqrt(rt, n2, tag):
            """rt = sqrt(n2) via exp(0.5*ln n2) — keeps ACT on the ln/exp
            activation table (no table reload between squash and softmax)."""
            p = n2.shape[0]
            lnn = smalls.tile(list(n2.shape), F32, tag="lnn", name=f"lnn{tag}")
            nc.scalar.activation(lnn, n2, mybir.ActivationFunctionType.Ln)
            nc.scalar.activation(rt, lnn, mybir.ActivationFunctionType.Exp, 0.0, 0.5)
            return rt

        def squash_wide(sf_sb, it):
            """squash on [128=(b,og), 256=(o8,j)] -> a (bf16) staged to DRAM."""
            s3 = sf_sb.rearrange("p (o j) -> p o j", j=OV)
            sq = smalls.tile([128, 8, OV], BF16, tag="sq", name=f"sq{it}")
            nc.vector.tensor_mul(out=sq, in0=s3, in1=s3)
            n2 = smalls.tile([128, 8], F32, tag="n2", name=f"n2{it}")
            nc.vector.reduce_sum(n2, sq, axis=mybir.AxisListType.X)
            rt = smalls.tile([128, 8], F32, tag="rt", name=f"rt{it}")
            ln_sqrt(rt, n2, f"s{it}")
            dn = smalls.tile([128, 8], F32, tag="dn", name=f"dn{it}")
            nc.vector.tensor_scalar_add(dn, n2, 1.0)
            nc.vector.reciprocal(dn, dn)
            f = smalls.tile([128, 8], F32, tag="f", name=f"f{it}")
            nc.vector.tensor_mul(out=f, in0=rt, in1=dn)
            a_sb = smalls.tile([128, 8, OV], BF16, tag="aw", name=f"aw{it}")
            nc.vector.tensor_mul(
                out=a_sb,
                in0=s3,
                in1=f[:, :, None].to_broadcast([128, 8, OV]),
            )
            a_dr = dpool.tile([128, 256], BF16, tag=f"adr{it}", name=f"adr{it}")
            af = a_sb.rearrange("p o j -> p (o j)")
            # stage per b-half so the bh0 broadcast (and F's first chunks)
            # can start while bh1 is still in flight
            nc.sync.dma_start(a_dr[0:64], af[0:64])
            nc.scalar.dma_start(a_dr[64:128], af[64:128])
            return a_dr

        def broadcast_a(a_dr):
            """arep2[(k8,bl), bh, (o,j)] = a[b=bh*16+bl, (o,j)], replicated
            over k8: one [16, 1024] DMA per (k8, bh) from the DRAM bounce
            (row b is a contiguous 2048B (o,j) run), bh0 first so F's first
            chunks start while bh1 is still broadcasting."""
            av = a_dr.rearrange("(h bl og) r -> h bl (og r)", h=2, og=4)
            dst = arep2.rearrange("(k8 bl) h f -> k8 bl h f", k8=8)
            qs = [nc.sync, nc.scalar, nc.gpsimd]
            for bh in range(2):
                for k8 in range(8):
                    qs[(bh * 8 + k8) % 3].dma_start(dst[k8, :, bh], av[bh])

        def fused_iteration(it):
            """lu(it) + softmax -> c(it+1) + ws(it+1), software-pipelined:
            stage A(i) = m6/tree/blog/exp/den-tree of chunk i;
            stage B1(i) = recip/den2 of chunk i; B2(i) = norm/m3/ws of i.
            Emission per index i: A(i), B1(i-1), B2(i-2) — so neither DVE nor
            Pool ever waits on the intra-chunk chain."""
            ps = pmix.tile([B, OJ], F32, tag="pm", name=f"ps{it}")
            nc.tensor.matmul(
                ps[0:2, 0:2],
                lhsT=dsumA_sb[0][:, 0:2],
                rhs=dsumA_sb[0][:, 0:2],
                start=True,
                stop=True,
            )
            NCH = 2 * NCC
            state = {}

            def stage_a(i):
                bh, cc = divmod(i, NCC)
                k0 = CH * cc
                u_c = u_sb[bh][:, k0 : k0 + CH]
                m6 = mprod.tile([128, CH, OJ], BF16, tag="prod", name="m6")
                m6_eng = nc.vector
                m6_eng.tensor_mul(
                    out=m6,
                    in0=u_c,
                    in1=arep2[:, bh, None, :].to_broadcast([128, CH, OJ]),
                )
                m6v = m6.rearrange("p c (o j) -> p c o j", j=OV)
                tt = trp.tile([128, CH, OC, 16], BF16, tag="tt", name="tt")
                nc.gpsimd.tensor_add(
                    out=tt, in0=m6v[..., 0:16], in1=m6v[..., 16:32]
                )
                nc.gpsimd.tensor_add(
                    out=tt[..., 0:8], in0=tt[..., 0:8], in1=tt[..., 8:16]
                )
                nc.gpsimd.tensor_add(
                    out=tt[..., 8:12], in0=tt[..., 0:4], in1=tt[..., 4:8]
                )
                nc.gpsimd.tensor_add(
                    out=tt[..., 12:14], in0=tt[..., 8:10], in1=tt[..., 10:12]
                )
                t5 = t5p.tile([128, CH, OC], BF16, tag="t5", name="t5")
                nc.gpsimd.tensor_add(
                    out=t5[:, :, :, None], in0=tt[..., 12:13], in1=tt[..., 13:14]
                )
                nc.gpsimd.tensor_add(
                    out=blog[bh][:, k0 : k0 + CH],
                    in0=blog[bh][:, k0 : k0 + CH],
                    in1=t5,
                )
                c3 = c3p.tile([128, CH, OC], BF16, tag="c3", name="c3")
                nc.scalar.activation(
                    c3,
                    blog[bh][:, k0 : k0 + CH],
                    mybir.ActivationFunctionType.Exp,
                )
                state[i] = (c3, None)  # den-tree deferred to b1

            def stage_b1(i):
                c3, _ = state.pop(i)
                dA = smp.tile([128, CH, 16], FP16, tag="dA", name="dA")
                nc.gpsimd.tensor_add(out=dA, in0=c3[..., 0:16], in1=c3[..., 16:32])
                nc.gpsimd.tensor_add(out=dA[..., 0:8], in0=dA[..., 0:8], in1=dA[..., 8:16])
                nc.gpsimd.tensor_add(out=dA[..., 8:12], in0=dA[..., 0:4], in1=dA[..., 4:8])
                nc.gpsimd.tensor_add(out=dA[..., 12:14], in0=dA[..., 8:10], in1=dA[..., 10:12])
                den = smp.tile([128, CH], F32, tag="den", name="den")
                nc.gpsimd.tensor_add(
                    out=den[:, :, None], in0=dA[..., 12:13], in1=dA[..., 13:14]
                )
                negln = smp.tile([128, CH], F32, tag="negln", name="negln")
                nc.scalar.activation(negln, den, mybir.ActivationFunctionType.Ln)
                nc.scalar.activation(
                    negln, negln, mybir.ActivationFunctionType.Copy, 0.0, -1.0
                )
                state[i] = negln

            def stage_b2_norm(i):
                bh, cc = divmod(i, NCC)
                k0 = CH * cc
                negln = state[i]
                c3e = cep.tile([128, CH, OC, 2], BF16, tag="c3e", name="c3e")
                for kgi in range(CH):
                    nc.scalar.activation(
                        c3e[:, kgi],
                        blog[bh][:, k0 + kgi, :, None].to_broadcast([128, OC, 2]),
                        mybir.ActivationFunctionType.Exp,
                        negln[:, kgi : kgi + 1],
                    )
                state[i] = c3e

            def stage_b2(i):
                bh, cc = divmod(i, NCC)
                k0 = CH * cc
                u_c = u_sb[bh][:, k0 : k0 + CH]
                c3e = state.pop(i)
                m3 = mprod.tile([128, CH, OC, OV], BF16, tag="prod", name="m3")
                m3_eng = nc.vector
                m3_eng.tensor_mul(
                    out=m3.rearrange("p c o (a b) -> p c o a b", b=2),
                    in0=u_c.rearrange("p c (o a b) -> p c o a b", o=OC, b=2),
                    in1=c3e[:, :, :, None, :].to_broadcast(
                        [128, CH, OC, OV // 2, 2]
                    ),
                )
                m3f = m3.rearrange("p c o j -> p c (o j)")
                for ci in range(CH):
                    for h in range(2):
                        nc.tensor.matmul(
                            ps[:, 512 * h : 512 * h + 512],
                            lhsT=dsumA_sb[bh],
                            rhs=m3f[:, ci, 512 * h : 512 * h + 512],
                            start=(i == 0 and ci == 0),
                            stop=(i == NCH - 1 and ci == CH - 1),
                        )

            for i in range(NCH + 2):
                if i >= 2:
                    stage_b2_norm(i - 2)
                if i < NCH:
                    stage_a(i)
                if 1 <= i <= NCH:
                    stage_b1(i - 1)
                if i >= 2:
                    stage_b2(i - 2)
            return ps

        # ---- routing ----
        sf0 = allreduce_s(s0ps, 0)
        emit_ring_b()
        a0 = squash_wide(sf0, 0)
        broadcast_a(a0)
        ps1 = fused_iteration(0)
        sf1 = allreduce_s(ps1, 1)
        a1 = squash_wide(sf1, 1)
        broadcast_a(a1)
        ps2 = fused_iteration(1)

        # ---- final: ReduceScatter s2, squash the local 4-sample slice ----
        s2_sb = smalls.tile([B, OJ], F32, tag="s_sb", name="s2_sb")
        nc.vector.tensor_copy(out=s2_sb, in_=ps2)
        sp2 = dpool.tile([B, OJ], F32, tag="sp2", name="sp2")
        sf2 = dpool.tile([B // NCORES, OJ], F32, tag="sf2", name="sf2")
        nc.gpsimd.dma_start(sp2, s2_sb)
        nc.gpsimd.collective_compute(
            "ReduceScatter",
            mybir.AluOpType.add,
            replica_groups=[list(range(NCORES))],
            ins=[sp2.opt()],
            outs=[sf2.opt()],
        )
        # [4, 1024] -> [32=(b,og8), 128=(o4,j)]
        sfv = smalls.tile([32, 128], F32, tag="sfv", name="sfv")
        nc.scalar.dma_start(sfv, sf2.rearrange("b (og r) -> (b og) r", og=8))
        s3 = sfv.rearrange("p (o j) -> p o j", j=OV)
        sq = smalls.tile([32, 4, OV], F32, tag="sqv", name="sqv")
        nc.vector.tensor_mul(out=sq, in0=s3, in1=s3)
        n2 = smalls.tile([32, 4], F32, tag="n2v", name="n2v")
        nc.vector.reduce_sum(n2, sq, axis=mybir.AxisListType.X)
        rt = smalls.tile([32, 4], F32, tag="rtv", name="rtv")
        ln_sqrt(rt, n2, "v")
        dn = smalls.tile([32, 4], F32, tag="dnv", name="dnv")
        nc.vector.tensor_scalar_add(dn, n2, 1.0)
        nc.vector.reciprocal(dn, dn)
        f = smalls.tile([32, 4], F32, tag="fv", name="fv")
        nc.vector.tensor_mul(out=f, in0=rt, in1=dn)
        vt = smalls.tile([32, 4, OV], F32, tag="vv", name="vv")
        nc.vector.tensor_mul(
            out=vt, in0=s3, in1=f[:, :, None].to_broadcast([32, 4, OV])
        )
        nc.gpsimd.dma_start(
            v_out.rearrange("b (og r) -> (b og) r", og=8),
            vt.rearrange("p o j -> p (o j)"),
        )


def _host_inputs(x, W):
    """Build per-core staged inputs (numpy, bf16) from full x [B,IC,IV], W [OC,IC,IV,OV]."""
    ins = []
    dsum0 = np.zeros((2, 128, 32), np.float32)
    dsumA = np.zeros((2, 128, 32), np.float32)
    for bh in range(2):
        for p in range(128):
            k8, bl = p // 16, p % 16
            dsum0[bh, p, 16 * bh + bl] = 1.0 / OC
            dsumA[bh, p, 16 * bh + bl] = 1.0
    dsum0 = dsum0.astype(_BF)
    dsumA = dsumA.astype(_BF)

    for c in range(NCORES):
        ksl = slice(KL * c, KL * (c + 1))
        Wc = np.ascontiguousarray(W[:, ksl])  # [o, 256, i, j]
        wr = (
            Wc.reshape(OC, KG, 8, IV, OV)
            .transpose(1, 2, 3, 0, 4)
            .reshape(KG, 128, OJ)
            .astype(_BF)
        )
        xc = np.ascontiguousarray(x[:, ksl])  # [32, 256, 16]
        xr = xc.reshape(2, 16, KG, 8, IV)  # [bh, bl, kg, k8, i]
        xb = np.zeros((8, IV, 2, KG, 8, 16), np.float32)  # [k8,i,bh,kg,k8',bl]
        for k8 in range(8):
            xb[k8, :, :, :, k8, :] = xr[:, :, :, k8, :].transpose(3, 0, 2, 1)
        xblk = xb.reshape(128, 2, KG, 128).astype(_BF)
        xsum = (
            (xr.transpose(3, 4, 2, 0, 1) / OC)  # [k8, i, kg, bh, bl]
            .reshape(128, KG, 32)
            .astype(_BF)
        )
        ins.append(
            {
                "wa": wr,
                "wb": wr,
                "xblk": xblk,
                "xsum": xsum,
                "dsum0": dsum0,
                "dsumA": dsumA,
            }
        )
    return ins


def kernel(x: np.ndarray, W: np.ndarray) -> np.ndarray:
    from concourse.bass_utils import run_bass_kernel_spmd

    x = np.asarray(x, np.float32)
    W = np.asarray(W, np.float32)
    nc = build_nc()
    in_maps = _host_inputs(x, W)
    res = run_bass_kernel_spmd(nc, in_maps, core_ids=list(range(NCORES)))
    parts = [
        res.results[c]["v"].reshape(B // NCORES, OC, OV).astype(np.float32)
        for c in range(NCORES)
    ]
    return np.concatenate(parts, axis=0)


if __name__ == "__main__":
    rng = np.random.default_rng(0)
    x = rng.standard_normal((B, IC, IV), dtype=np.float32)
    W = (0.01 * rng.standard_normal((OC, IC, IV, OV))).astype(np.float32)
    v = kernel(x, W)
    print("v", v.shape, v.dtype, float(np.abs(v).max()))
